# revision 8
# baseline (speedup 1.0000x reference)
"""GCN (2-layer, PyG GCNConv semantics) on 8 Trainium2 NeuronCores.

Strategy (dst-shard, graph-parallel):
- Nodes are sharded contiguously across the 8 cores (12500 dsts/core).
- All dense math runs on-device via Bass/Tile in 3 SPMD dispatches:
    A: h1 = x @ W1           (x shipped pre-transposed in bf16, PE matmuls)
    B: s1 = segment-sum of gathered u1 rows over dst groups (PE staircase
       one-hot matmuls built on-device from per-slot dst offsets), fused
       epilogue -> relu1, v2 = dinv*relu1
    C: same segment-sum machinery for layer 2, then @W2 + b2 + log_softmax
- The edge structure (sort order, slot layout, staircase metadata) is
  compile-time constant: it is baked into the instruction stream / tiny
  static inputs at kernel-build time.
- The two per-edge value gathers (u[src] for 3.2M edges) run on the host
  between dispatches: every data-driven gather primitive available in this
  toolchain was measured unusable (indirect DMA ~1.6us/row and 128 rows per
  call; GPSIMD gather ucode unloadable under this walrus build).
"""
import os
import sys
import numpy as np

sys.path.insert(0, "/opt/trn_rl_repo")

import ml_dtypes
import concourse.bass as bass
import concourse.mybir as mybir
import concourse.tile as tile
from concourse.vector_clock import ScopedClock
from concourse.bass_utils import run_bass_kernel_spmd

BF16 = mybir.dt.bfloat16
F32 = mybir.dt.float32
AF = mybir.ActivationFunctionType
ALU = mybir.AluOpType
NPBF16 = ml_dtypes.bfloat16

N_CORES = 8
GROUP = 32          # dsts per staircase group (matmul M)
SC = 64             # chunks per superchunk (is_equal batch)

# ---------------------------------------------------------------------------
# walrus workaround: only ONE sync-wait command per instruction is accepted.
# ---------------------------------------------------------------------------


def _patched_drain_and_barrier(self, tick_clock, wait_clock):
    nc = self.nc
    carrier = nc.sync.nop(nofuse=True, hint="drain_wait_carrier")
    wait_clock.add_sem_waits(carrier.ins, ScopedClock({None: tick_clock.global_clock}))
    si = carrier.ins.sync_info
    waits = list(si.on_wait or []) if si else []
    if len(waits) > 1:
        si.on_wait = waits[:1]
        for i in range(1, len(waits)):
            extra = nc.sync.nop(nofuse=True, hint="drain_wait_carrier")
            extra.ins.sync_info = mybir.SyncInfo(on_wait=waits[i : i + 1], on_update=[])
    nc.sync.drain()
    nc.all_engine_barrier()
    assert self.sems is not None
    popped = nc._tile_sem_poison_stack.pop()
    assert popped is self._sem_poison
    nc.clear_and_free_semaphores(list(self.sems.allocated().values()))
    nc.all_engine_barrier()


tile.TileContext._drain_and_barrier = _patched_drain_and_barrier


def _legalize_waits(nc, max_waits=1):
    n = [0]

    def mk_nop(engine, waits):
        n[0] += 1
        return mybir.InstNoOp(
            name=f"waitnop-{n[0]}",
            engine=engine,
            ins=[],
            outs=[],
            sync_info=mybir.SyncInfo(on_wait=list(waits), on_update=[]),
            text_hint="wait_carrier",
        )

    for f in nc.m.functions:
        for bb in f.blocks:
            out = []
            changed = False
            for inst in bb.instructions:
                si = inst.sync_info
                waits = list(si.on_wait or []) if si else []
                if len(waits) > max_waits:
                    changed = True
                    for i in range(0, len(waits) - max_waits, max_waits):
                        out.append(mk_nop(inst.engine, waits[i : i + max_waits]))
                    si.on_wait = waits[len(waits) - max_waits :]
                out.append(inst)
            if changed:
                bb.instructions = out


# ---------------------------------------------------------------------------
# device kernel builders
# ---------------------------------------------------------------------------


def build_A(NT, FIN=512):
    """h1 = x @ W1 per core. xTr host layout [128, FIN//128, NT*128] bf16."""
    FC = FIN // 128
    nc = bass.Bass()
    xT = nc.dram_tensor("xT", [128, FC, NT * 128], BF16, kind="ExternalInput")
    W1b = nc.dram_tensor("W1b", [128, FC, 16], BF16, kind="ExternalInput")
    h1 = nc.dram_tensor("h1", [NT * 128, 16], F32, kind="ExternalOutput")
    with tile.TileContext(nc) as tc:
        with (
            tc.tile_pool(name="stat", bufs=1) as spool,
            tc.tile_pool(name="psum", bufs=8, space="PSUM") as pp,
        ):
            w1 = spool.tile([128, FC, 16], BF16)
            nc.sync.dma_start(out=w1[:], in_=W1b[:])
            # one whole-tensor load: per-partition contiguous FC*NT*128*2B run
            # (per-tile loads move 256B/partition chunks -> 2x sub-512B DMA
            # penalty; this is one big descriptor per partition instead)
            xsb = spool.tile([128, FC, NT * 128], BF16)
            nc.sync.dma_start(out=xsb[:], in_=xT[:])
            h_sb = spool.tile([128, NT, 16], F32)
            for t in range(NT):
                ps = pp.tile([128, 16], F32, tag="hps")
                for fc in range(FC):
                    nc.tensor.matmul(
                        out=ps[:],
                        lhsT=xsb[:, fc, 128 * t : 128 * (t + 1)],
                        rhs=w1[:, fc, :],
                        start=(fc == 0),
                        stop=(fc == FC - 1),
                    )
                nc.scalar.copy(out=h_sb[:, t, :], in_=ps[:])
            nc.sync.dma_start(
                out=h1.rearrange("(t p) f -> p t f", p=128), in_=h_sb[:]
            )
    _legalize_waits(nc)
    return nc


def _emit_segsum(nc, tc, pool, spool, pp, g_dram, dstid_sb, iota_sb, chunks, s_sb, nchunks):
    """Staircase segment-sum: s_sb[128, NT, 16] f32 <- sum of g rows per dst."""
    nsc = (nchunks + SC - 1) // SC
    ps = None
    for sc in range(nsc):
        cs = sc * SC
        w = min(SC, nchunks - cs)
        g_sc = pool.tile([128, SC, 16], BF16, tag="gsc")
        nc.sync.dma_start(out=g_sc[:, :w, :], in_=g_dram[:, cs : cs + w, :])
        s_all = pool.tile([128, SC, GROUP], BF16, tag="sall")
        nc.vector.tensor_tensor(
            out=s_all[:, :w, :],
            in0=dstid_sb[:, cs : cs + w].to_broadcast([128, w, GROUP]),
            in1=iota_sb[:, :w, :],
            op=ALU.is_equal,
        )
        for j in range(w):
            grp, st, sp = chunks[cs + j]
            if st:
                ps = pp.tile([GROUP, 16], F32, tag="ps")
            nc.tensor.matmul(
                out=ps[:],
                lhsT=s_all[:, j, :],
                rhs=g_sc[:, j, :],
                start=st,
                stop=sp,
            )
            if sp:
                po = GROUP * (grp % (128 // GROUP))
                nc.scalar.copy(
                    out=s_sb[po : po + GROUP, grp // (128 // GROUP), :], in_=ps[:]
                )


def build_B(NT, nchunks, chunks):
    """s1 -> agg1 -> relu1, v2."""
    nc = bass.Bass()
    g = nc.dram_tensor("g", [128, nchunks, 16], BF16, kind="ExternalInput")
    dstid = nc.dram_tensor("dstid", [128, nchunks], BF16, kind="ExternalInput")
    iota = nc.dram_tensor("iota", [128, SC, GROUP], BF16, kind="ExternalInput")
    h1 = nc.dram_tensor("h1", [NT * 128, 16], F32, kind="ExternalInput")
    dinva = nc.dram_tensor("dinva", [128, NT], F32, kind="ExternalInput")
    dinv2a = nc.dram_tensor("dinv2a", [128, NT], F32, kind="ExternalInput")
    b1rep = nc.dram_tensor("b1rep", [128, NT, 16], F32, kind="ExternalInput")
    relu1 = nc.dram_tensor("relu1", [NT * 128, 16], F32, kind="ExternalOutput")
    v2 = nc.dram_tensor("v2", [NT * 128, 16], BF16, kind="ExternalOutput")
    with tile.TileContext(nc) as tc:
        with (
            tc.tile_pool(name="sbuf", bufs=2) as pool,
            tc.tile_pool(name="stat", bufs=1) as spool,
            tc.tile_pool(name="psum", bufs=8, space="PSUM") as pp,
        ):
            dstid_sb = spool.tile([128, nchunks], BF16)
            nc.sync.dma_start(out=dstid_sb[:], in_=dstid[:])
            iota_sb = spool.tile([128, SC, GROUP], BF16)
            nc.sync.dma_start(out=iota_sb[:], in_=iota[:])
            h1_sb = spool.tile([128, NT, 16], F32)
            nc.sync.dma_start(out=h1_sb[:], in_=h1.rearrange("(t p) f -> p t f", p=128))
            dinva_sb = spool.tile([128, NT], F32)
            nc.sync.dma_start(out=dinva_sb[:], in_=dinva[:])
            dinv2a_sb = spool.tile([128, NT], F32)
            nc.sync.dma_start(out=dinv2a_sb[:], in_=dinv2a[:])
            b1_sb = spool.tile([128, NT, 16], F32)
            nc.sync.dma_start(out=b1_sb[:], in_=b1rep[:])
            s_sb = spool.tile([128, NT, 16], F32)

            _emit_segsum(nc, tc, pool, spool, pp, g, dstid_sb, iota_sb, chunks, s_sb, nchunks)

            tmp = spool.tile([128, NT, 16], F32)
            tmp2 = spool.tile([128, NT, 16], F32)
            nc.vector.tensor_tensor(
                out=tmp[:], in0=s_sb[:], in1=dinva_sb[:].to_broadcast([128, NT, 16]),
                op=ALU.mult,
            )
            nc.vector.tensor_tensor(
                out=tmp2[:], in0=h1_sb[:], in1=dinv2a_sb[:].to_broadcast([128, NT, 16]),
                op=ALU.mult,
            )
            nc.vector.tensor_tensor(out=tmp[:], in0=tmp[:], in1=tmp2[:], op=ALU.add)
            nc.vector.tensor_tensor(out=tmp[:], in0=tmp[:], in1=b1_sb[:], op=ALU.add)
            relu_sb = spool.tile([128, NT, 16], F32)
            nc.scalar.activation(out=relu_sb[:], in_=tmp[:], func=AF.Relu)
            v2_sb = spool.tile([128, NT, 16], BF16)
            nc.vector.tensor_tensor(
                out=v2_sb[:], in0=relu_sb[:],
                in1=dinva_sb[:].to_broadcast([128, NT, 16]), op=ALU.mult,
            )
            nc.sync.dma_start(
                out=relu1.rearrange("(t p) f -> p t f", p=128), in_=relu_sb[:]
            )
            nc.sync.dma_start(out=v2.rearrange("(t p) f -> p t f", p=128), in_=v2_sb[:])
    _legalize_waits(nc)
    return nc


def build_C(NT, nchunks, chunks):
    """s2 -> agg2 -> @W2 + b2 -> log_softmax."""
    nc = bass.Bass()
    g = nc.dram_tensor("g", [128, nchunks, 16], BF16, kind="ExternalInput")
    dstid = nc.dram_tensor("dstid", [128, nchunks], BF16, kind="ExternalInput")
    iota = nc.dram_tensor("iota", [128, SC, GROUP], BF16, kind="ExternalInput")
    relu1 = nc.dram_tensor("relu1", [NT * 128, 16], F32, kind="ExternalInput")
    dinva = nc.dram_tensor("dinva", [128, NT], F32, kind="ExternalInput")
    dinv2a = nc.dram_tensor("dinv2a", [128, NT], F32, kind="ExternalInput")
    b2rep = nc.dram_tensor("b2rep", [128, NT, 16], F32, kind="ExternalInput")
    ident = nc.dram_tensor("ident", [128, 128], F32, kind="ExternalInput")
    W2b = nc.dram_tensor("W2b", [16, 16], BF16, kind="ExternalInput")
    outd = nc.dram_tensor("outd", [NT * 128, 16], F32, kind="ExternalOutput")
    with tile.TileContext(nc) as tc:
        with (
            tc.tile_pool(name="sbuf", bufs=2) as pool,
            tc.tile_pool(name="stat", bufs=1) as spool,
            tc.tile_pool(name="psum", bufs=4, space="PSUM") as pp,
            tc.tile_pool(name="psumt", bufs=2, space="PSUM") as ppt,
        ):
            dstid_sb = spool.tile([128, nchunks], BF16)
            nc.sync.dma_start(out=dstid_sb[:], in_=dstid[:])
            iota_sb = spool.tile([128, SC, GROUP], BF16)
            nc.sync.dma_start(out=iota_sb[:], in_=iota[:])
            r1_sb = spool.tile([128, NT, 16], F32)
            nc.sync.dma_start(
                out=r1_sb[:], in_=relu1.rearrange("(t p) f -> p t f", p=128)
            )
            dinva_sb = spool.tile([128, NT], F32)
            nc.sync.dma_start(out=dinva_sb[:], in_=dinva[:])
            dinv2a_sb = spool.tile([128, NT], F32)
            nc.sync.dma_start(out=dinv2a_sb[:], in_=dinv2a[:])
            b2_sb = spool.tile([128, NT, 16], F32)
            nc.sync.dma_start(out=b2_sb[:], in_=b2rep[:])
            id_sb = spool.tile([128, 128], F32)
            nc.sync.dma_start(out=id_sb[:], in_=ident[:])
            w2_sb = spool.tile([16, 16], BF16)
            nc.sync.dma_start(out=w2_sb[:], in_=W2b[:])
            s_sb = spool.tile([128, NT, 16], F32)

            _emit_segsum(nc, tc, pool, spool, pp, g, dstid_sb, iota_sb, chunks, s_sb, nchunks)

            agg = spool.tile([128, NT, 16], F32)
            tmp2 = spool.tile([128, NT, 16], F32)
            nc.vector.tensor_tensor(
                out=agg[:], in0=s_sb[:], in1=dinva_sb[:].to_broadcast([128, NT, 16]),
                op=ALU.mult,
            )
            nc.vector.tensor_tensor(
                out=tmp2[:], in0=r1_sb[:], in1=dinv2a_sb[:].to_broadcast([128, NT, 16]),
                op=ALU.mult,
            )
            nc.vector.tensor_tensor(out=agg[:], in0=agg[:], in1=tmp2[:], op=ALU.add)

            z_sb = spool.tile([128, NT, 16], F32)
            for tb in range(0, NT, 4):
                w4 = min(4, NT - tb)
                trps = ppt.tile([16, 512], F32, tag="trps")
                for k in range(w4):
                    nc.tensor.transpose(
                        out=trps[:, 128 * k : 128 * (k + 1)],
                        in_=agg[:, tb + k, :],
                        identity=id_sb[:],
                    )
                aggT = pool.tile([16, 512], BF16, tag="aggT")
                nc.scalar.copy(out=aggT[:, : 128 * w4], in_=trps[:, : 128 * w4])
                for k in range(w4):
                    zps = ppt.tile([128, 16], F32, tag="zps")
                    nc.tensor.matmul(
                        out=zps[:],
                        lhsT=aggT[:, 128 * k : 128 * (k + 1)],
                        rhs=w2_sb[:],
                        start=True,
                        stop=True,
                    )
                    nc.scalar.copy(out=z_sb[:, tb + k, :], in_=zps[:])

            nc.vector.tensor_tensor(out=z_sb[:], in0=z_sb[:], in1=b2_sb[:], op=ALU.add)
            m_sb = spool.tile([128, NT], F32)
            nc.vector.tensor_reduce(
                out=m_sb[:], in_=z_sb[:], axis=mybir.AxisListType.X, op=ALU.max
            )
            zc = spool.tile([128, NT, 16], F32)
            nc.vector.tensor_tensor(
                out=zc[:], in0=z_sb[:], in1=m_sb[:].to_broadcast([128, NT, 16]),
                op=ALU.subtract,
            )
            e_sb = spool.tile([128, NT, 16], F32)
            nc.scalar.activation(out=e_sb[:], in_=zc[:], func=AF.Exp)
            ss = spool.tile([128, NT], F32)
            nc.vector.tensor_reduce(
                out=ss[:], in_=e_sb[:], axis=mybir.AxisListType.X, op=ALU.add
            )
            lse = spool.tile([128, NT], F32)
            nc.scalar.activation(out=lse[:], in_=ss[:], func=AF.Ln)
            o_sb = spool.tile([128, NT, 16], F32)
            nc.vector.tensor_tensor(
                out=o_sb[:], in0=zc[:], in1=lse[:].to_broadcast([128, NT, 16]),
                op=ALU.subtract,
            )
            nc.sync.dma_start(out=outd.rearrange("(t p) f -> p t f", p=128), in_=o_sb[:])
    _legalize_waits(nc)
    return nc


# ---------------------------------------------------------------------------
# host side
# ---------------------------------------------------------------------------


def _preprocess(edge_index, n_nodes, per_core):
    """Sort edges by dst, build common-across-cores slot/chunk structure."""
    src = np.asarray(edge_index[0])
    dst = np.asarray(edge_index[1])
    deg = np.bincount(dst, minlength=n_nodes).astype(np.float32) + 1.0
    dinv = 1.0 / np.sqrt(deg)

    order = np.argsort(dst, kind="stable")
    sdst = dst[order]
    ssrc = src[order]

    NT = (per_core + 127) // 128
    padded = NT * 128
    ngroups = padded // GROUP

    bounds = np.searchsorted(sdst, np.arange(N_CORES + 1) * per_core)
    core_grp_cnt = np.zeros((N_CORES, ngroups), np.int64)
    core_edges = []
    for c in range(N_CORES):
        lo, hi = bounds[c], bounds[c + 1]
        ld = sdst[lo:hi] - c * per_core
        grp = ld >> 5
        core_grp_cnt[c] = np.bincount(grp, minlength=ngroups)
        core_edges.append((ld, ssrc[lo:hi]))

    nchunk_g = np.maximum((core_grp_cnt.max(axis=0) + 127) // 128, 1)
    chunk_base = np.concatenate([[0], np.cumsum(nchunk_g)])
    nchunks = int(chunk_base[-1])
    # pad nchunks to a multiple of 4 for tidiness
    chunks = []
    for gi in range(ngroups):
        for k in range(nchunk_g[gi]):
            chunks.append((gi, k == 0, k == nchunk_g[gi] - 1))

    dstid_arrs, sidx_arrs = [], []
    for c in range(N_CORES):
        ld, esrc = core_edges[c]
        grp = ld >> 5
        # rank of each edge within its group (edges sorted by dst => grouped)
        gstart = np.concatenate([[0], np.cumsum(core_grp_cnt[c])])
        rank = np.arange(len(ld)) - np.repeat(gstart[:-1], core_grp_cnt[c])
        slot = chunk_base[grp] * 128 + rank
        nslots = nchunks * 128
        dstid_slots = np.full(nslots, -1.0, np.float32)
        dstid_slots[slot] = (ld & 31).astype(np.float32)
        sidx_slots = np.zeros(nslots, np.int64)
        sidx_slots[slot] = esrc
        dstid_arrs.append(
            dstid_slots.reshape(nchunks, 128).T.astype(NPBF16).copy()
        )
        sidx_arrs.append(sidx_slots.reshape(nchunks, 128).T.copy())
    return dinv, NT, nchunks, chunks, dstid_arrs, sidx_arrs


_CACHE = {}
LAST_TIMES = {}
LAST_HW_NS = None
_TRACE = bool(os.environ.get("KERNEL_TRACE"))


def _sim_ns(nc):
    """Cost-model (CoreSim no-exec) execution time of one dispatch, ns."""
    from concourse.bass_interp import CoreSim

    sim = CoreSim(nc, no_exec=True)
    sim.simulate()
    return int(sim.time)


def _run(nc, in_maps, cores, tag):
    import time as _t

    global LAST_HW_NS
    t0 = _t.time()
    res = run_bass_kernel_spmd(nc, in_maps, core_ids=cores, trace=_TRACE)
    LAST_TIMES[f"disp_{tag}"] = _t.time() - t0
    if res.exec_time_ns is not None:
        LAST_TIMES[f"hw_{tag}_ns"] = res.exec_time_ns
        LAST_HW_NS = (LAST_HW_NS or 0) + res.exec_time_ns
    return res


def _kernel_impl(x, W1, b1, W2, b2, edge_index, n_nodes, per_core):
    x = np.asarray(x, dtype=np.float32)
    W1 = np.asarray(W1, dtype=np.float32)
    b1 = np.asarray(b1, dtype=np.float32)
    W2 = np.asarray(W2, dtype=np.float32)
    b2 = np.asarray(b2, dtype=np.float32)
    edge_index = np.asarray(edge_index)
    fin = x.shape[1]

    import time as _t
    LAST_TIMES.clear()
    _tp = _t.time()
    dinv, NT, nchunks, chunks, dstid_arrs, sidx_arrs = _preprocess(
        edge_index, n_nodes, per_core
    )
    LAST_TIMES["preprocess"] = _t.time() - _tp
    padded = NT * 128
    cores = list(range(N_CORES))

    key = (n_nodes, per_core, nchunks)
    if key not in _CACHE:
        ncA = build_A(NT, fin)
        ncB = build_B(NT, nchunks, chunks)
        ncC = build_C(NT, nchunks, chunks)
        try:
            hw_ns = _sim_ns(ncA) + _sim_ns(ncB) + _sim_ns(ncC)
        except Exception:
            hw_ns = None
        _CACHE[key] = (ncA, ncB, ncC, hw_ns)
    ncA, ncB, ncC, _hw = _CACHE[key]
    global LAST_HW_NS
    LAST_HW_NS = _hw
    LAST_TIMES["build"] = _t.time() - _tp

    # ---- dispatch A: h1 = x @ W1 ----
    FC = fin // 128
    W1bf = W1.astype(NPBF16)
    W1r = W1bf.reshape(FC, 128, 16).transpose(1, 0, 2).copy()  # [128, FC, 16]
    in_A = []
    for c in cores:
        xs = x[c * per_core : (c + 1) * per_core]
        xp = np.zeros((padded, fin), np.float32)
        xp[: xs.shape[0]] = xs
        xTr = (
            xp.T.astype(NPBF16).reshape(FC, 128, padded).transpose(1, 0, 2).copy()
        )  # [128, FC, padded]
        in_A.append({"xT": xTr, "W1b": W1r})
    LAST_TIMES["prep_A"] = _t.time() - _tp
    resA = _run(ncA, in_A, cores, "A")
    h1s = [resA.results[c]["h1"] for c in cores]  # [padded, 16] f32

    # ---- host gather for layer 1 ----
    _tp = _t.time()
    u1 = np.concatenate([h1s[c][:per_core] for c in cores], axis=0)
    u1 *= dinv[:, None]

    # static scale/bias arrays per core
    iota_np = np.broadcast_to(
        np.arange(GROUP, dtype=np.float32), (128, SC, GROUP)
    ).astype(NPBF16)
    ident_np = np.eye(128, dtype=np.float32)
    W2bf = W2.astype(NPBF16)
    dinva_c, dinv2a_c, b1rep, b2rep = [], [], None, None
    for c in cores:
        dv = np.ones(padded, np.float32)
        dv[:per_core] = dinv[c * per_core : (c + 1) * per_core]
        dinva_c.append(dv.reshape(NT, 128).T.copy())
        dinv2a_c.append((dv * dv).reshape(NT, 128).T.copy())
    b1rep = np.broadcast_to(b1, (128, NT, 16)).astype(np.float32).copy()
    b2rep = np.broadcast_to(b2, (128, NT, 16)).astype(np.float32).copy()

    def gath(table, c):
        return table[sidx_arrs[c]].astype(NPBF16)  # [128, nchunks, 16]

    # ---- dispatch B ----
    in_B = []
    for c in cores:
        in_B.append(
            {
                "g": gath(u1, c),
                "dstid": dstid_arrs[c],
                "iota": iota_np,
                "h1": h1s[c],
                "dinva": dinva_c[c],
                "dinv2a": dinv2a_c[c],
                "b1rep": b1rep,
            }
        )
    LAST_TIMES["prep_B"] = _t.time() - _tp
    resB = _run(ncB, in_B, cores, "B")
    relu1s = [resB.results[c]["relu1"] for c in cores]
    v2s = [resB.results[c]["v2"] for c in cores]

    # ---- host gather for layer 2 ----
    _tp = _t.time()
    v2full = np.concatenate(
        [v2s[c][:per_core].astype(np.float32) for c in cores], axis=0
    )

    # ---- dispatch C ----
    in_C = []
    for c in cores:
        in_C.append(
            {
                "g": gath(v2full, c),
                "dstid": dstid_arrs[c],
                "iota": iota_np,
                "relu1": relu1s[c],
                "dinva": dinva_c[c],
                "dinv2a": dinv2a_c[c],
                "b2rep": b2rep,
                "ident": ident_np,
                "W2b": W2bf,
            }
        )
    LAST_TIMES["prep_C"] = _t.time() - _tp
    resC = _run(ncC, in_C, cores, "C")
    out = np.concatenate(
        [resC.results[c]["outd"][:per_core] for c in cores], axis=0
    ).astype(np.float32)
    return out


def kernel(x, W1, b1, W2, b2, edge_index):
    return _kernel_impl(x, W1, b1, W2, b2, edge_index, 100000, 12500)



# revision 9
# speedup vs baseline: 1.2425x; 1.2425x over previous
"""GCN (2-layer, PyG GCNConv semantics) on 8 Trainium2 NeuronCores.

Strategy (dst-shard, graph-parallel):
- Nodes are sharded contiguously across the 8 cores (12500 dsts/core).
- All dense math runs on-device via Bass/Tile in 3 SPMD dispatches:
    A: h1 = x @ W1           (x shipped pre-transposed in bf16, PE matmuls)
    B: s1 = segment-sum of gathered u1 rows over dst groups (PE staircase
       one-hot matmuls built on-device from per-slot dst offsets), fused
       epilogue -> relu1, v2 = dinv*relu1
    C: same segment-sum machinery for layer 2, then @W2 + b2 + log_softmax
- The edge structure (sort order, slot layout, staircase metadata) is
  compile-time constant: it is baked into the instruction stream / tiny
  static inputs at kernel-build time.
- The two per-edge value gathers (u[src] for 3.2M edges) run on the host
  between dispatches: every data-driven gather primitive available in this
  toolchain was measured unusable (indirect DMA ~1.6us/row and 128 rows per
  call; GPSIMD gather ucode unloadable under this walrus build).
"""
import os
import sys
import numpy as np

sys.path.insert(0, "/opt/trn_rl_repo")

import ml_dtypes
import concourse.bass as bass
import concourse.mybir as mybir
import concourse.tile as tile
from concourse.vector_clock import ScopedClock
from concourse.bass_utils import run_bass_kernel_spmd

BF16 = mybir.dt.bfloat16
F32 = mybir.dt.float32
AF = mybir.ActivationFunctionType
ALU = mybir.AluOpType
NPBF16 = ml_dtypes.bfloat16

N_CORES = 8
GROUP = 32          # dsts per staircase group (matmul M)
SC = 64             # chunks per superchunk (is_equal batch)

# ---------------------------------------------------------------------------
# walrus workaround: only ONE sync-wait command per instruction is accepted.
# ---------------------------------------------------------------------------


def _patched_drain_and_barrier(self, tick_clock, wait_clock):
    nc = self.nc
    carrier = nc.sync.nop(nofuse=True, hint="drain_wait_carrier")
    wait_clock.add_sem_waits(carrier.ins, ScopedClock({None: tick_clock.global_clock}))
    si = carrier.ins.sync_info
    waits = list(si.on_wait or []) if si else []
    if len(waits) > 1:
        si.on_wait = waits[:1]
        for i in range(1, len(waits)):
            extra = nc.sync.nop(nofuse=True, hint="drain_wait_carrier")
            extra.ins.sync_info = mybir.SyncInfo(on_wait=waits[i : i + 1], on_update=[])
    nc.sync.drain()
    nc.all_engine_barrier()
    assert self.sems is not None
    popped = nc._tile_sem_poison_stack.pop()
    assert popped is self._sem_poison
    nc.clear_and_free_semaphores(list(self.sems.allocated().values()))
    nc.all_engine_barrier()


tile.TileContext._drain_and_barrier = _patched_drain_and_barrier


def _legalize_waits(nc, max_waits=1):
    n = [0]

    def mk_nop(engine, waits):
        n[0] += 1
        return mybir.InstNoOp(
            name=f"waitnop-{n[0]}",
            engine=engine,
            ins=[],
            outs=[],
            sync_info=mybir.SyncInfo(on_wait=list(waits), on_update=[]),
            text_hint="wait_carrier",
        )

    for f in nc.m.functions:
        for bb in f.blocks:
            out = []
            changed = False
            for inst in bb.instructions:
                si = inst.sync_info
                waits = list(si.on_wait or []) if si else []
                if len(waits) > max_waits:
                    changed = True
                    for i in range(0, len(waits) - max_waits, max_waits):
                        out.append(mk_nop(inst.engine, waits[i : i + max_waits]))
                    si.on_wait = waits[len(waits) - max_waits :]
                out.append(inst)
            if changed:
                bb.instructions = out


# ---------------------------------------------------------------------------
# device kernel builders
# ---------------------------------------------------------------------------


def build_A(NT, FIN=512):
    """h1 = x @ W1 per core. xTr host layout [128, FIN//128, NT*128] bf16."""
    FC = FIN // 128
    nc = bass.Bass()
    xT = nc.dram_tensor("xT", [128, FC, NT * 128], BF16, kind="ExternalInput")
    W1b = nc.dram_tensor("W1b", [128, FC, 16], BF16, kind="ExternalInput")
    h1 = nc.dram_tensor("h1", [NT * 128, 16], F32, kind="ExternalOutput")
    with tile.TileContext(nc) as tc:
        with (
            tc.tile_pool(name="stat", bufs=1) as spool,
            tc.tile_pool(name="psum", bufs=8, space="PSUM") as pp,
        ):
            w1 = spool.tile([128, FC, 16], BF16)
            nc.sync.dma_start(out=w1[:], in_=W1b[:])
            # one whole-tensor load: per-partition contiguous FC*NT*128*2B run
            # (per-tile loads move 256B/partition chunks -> 2x sub-512B DMA
            # penalty; this is one big descriptor per partition instead)
            xsb = spool.tile([128, FC, NT * 128], BF16)
            nc.sync.dma_start(out=xsb[:], in_=xT[:])
            h_sb = spool.tile([128, NT, 16], F32)
            for t in range(NT):
                ps = pp.tile([128, 16], F32, tag="hps")
                for fc in range(FC):
                    nc.tensor.matmul(
                        out=ps[:],
                        lhsT=xsb[:, fc, 128 * t : 128 * (t + 1)],
                        rhs=w1[:, fc, :],
                        start=(fc == 0),
                        stop=(fc == FC - 1),
                    )
                nc.scalar.copy(out=h_sb[:, t, :], in_=ps[:])
            nc.sync.dma_start(
                out=h1.rearrange("(t p) f -> p t f", p=128), in_=h_sb[:]
            )
    _legalize_waits(nc)
    return nc


def _emit_segsum(nc, tc, pool, spool, pp, g_dram, dstid_sb, iota_sb, chunks, s_sb, nchunks):
    """Staircase segment-sum: s_sb[128, NT, 16] f32 <- sum of g rows per dst."""
    nsc = (nchunks + SC - 1) // SC
    ps = None
    for sc in range(nsc):
        cs = sc * SC
        w = min(SC, nchunks - cs)
        g_sc = pool.tile([128, SC, 16], BF16, tag="gsc")
        nc.sync.dma_start(out=g_sc[:, :w, :], in_=g_dram[:, cs : cs + w, :])
        # d-major one-hot [128, GROUP, w]: every operand's last dim is packed
        # 2-byte, so the DVE runs this in its 2x perf mode (the old chunk-major
        # layout put the broadcast on the last dim, forcing full-rate).
        s_all = pool.tile([128, GROUP, SC], BF16, tag="sall")
        nc.vector.tensor_tensor(
            out=s_all[:, :, :w],
            in0=dstid_sb[:, cs : cs + w]
            .rearrange("p (o j) -> p o j", o=1)
            .to_broadcast([128, GROUP, w]),
            in1=iota_sb[:, :, :w],
            op=ALU.is_equal,
        )
        for j in range(w):
            grp, st, sp = chunks[cs + j]
            if st:
                ps = pp.tile([GROUP, 16], F32, tag="ps")
            nc.tensor.matmul(
                out=ps[:],
                lhsT=s_all[:, :, j],
                rhs=g_sc[:, j, :],
                start=st,
                stop=sp,
            )
            if sp:
                po = GROUP * (grp % (128 // GROUP))
                nc.scalar.copy(
                    out=s_sb[po : po + GROUP, grp // (128 // GROUP), :], in_=ps[:]
                )


def build_B(NT, nchunks, chunks):
    """s1 -> agg1 -> relu1, v2."""
    nc = bass.Bass()
    g = nc.dram_tensor("g", [128, nchunks, 16], BF16, kind="ExternalInput")
    dstid = nc.dram_tensor("dstid", [128, nchunks], BF16, kind="ExternalInput")
    iota = nc.dram_tensor("iota", [128, GROUP, SC], BF16, kind="ExternalInput")
    h1 = nc.dram_tensor("h1", [NT * 128, 16], F32, kind="ExternalInput")
    dinva = nc.dram_tensor("dinva", [128, NT], F32, kind="ExternalInput")
    dinv2a = nc.dram_tensor("dinv2a", [128, NT], F32, kind="ExternalInput")
    b1rep = nc.dram_tensor("b1rep", [128, NT, 16], F32, kind="ExternalInput")
    relu1 = nc.dram_tensor("relu1", [NT * 128, 16], F32, kind="ExternalOutput")
    v2 = nc.dram_tensor("v2", [NT * 128, 16], BF16, kind="ExternalOutput")
    with tile.TileContext(nc) as tc:
        with (
            tc.tile_pool(name="sbuf", bufs=2) as pool,
            tc.tile_pool(name="stat", bufs=1) as spool,
            tc.tile_pool(name="psum", bufs=8, space="PSUM") as pp,
        ):
            dstid_sb = spool.tile([128, nchunks], BF16)
            nc.sync.dma_start(out=dstid_sb[:], in_=dstid[:])
            iota_sb = spool.tile([128, GROUP, SC], BF16)
            nc.sync.dma_start(out=iota_sb[:], in_=iota[:])
            h1_sb = spool.tile([128, NT, 16], F32)
            nc.sync.dma_start(out=h1_sb[:], in_=h1.rearrange("(t p) f -> p t f", p=128))
            dinva_sb = spool.tile([128, NT], F32)
            nc.sync.dma_start(out=dinva_sb[:], in_=dinva[:])
            dinv2a_sb = spool.tile([128, NT], F32)
            nc.sync.dma_start(out=dinv2a_sb[:], in_=dinv2a[:])
            b1_sb = spool.tile([128, NT, 16], F32)
            nc.sync.dma_start(out=b1_sb[:], in_=b1rep[:])
            s_sb = spool.tile([128, NT, 16], F32)

            _emit_segsum(nc, tc, pool, spool, pp, g, dstid_sb, iota_sb, chunks, s_sb, nchunks)

            tmp = spool.tile([128, NT, 16], F32)
            tmp2 = spool.tile([128, NT, 16], F32)
            nc.vector.tensor_tensor(
                out=tmp[:], in0=s_sb[:], in1=dinva_sb[:].to_broadcast([128, NT, 16]),
                op=ALU.mult,
            )
            nc.vector.tensor_tensor(
                out=tmp2[:], in0=h1_sb[:], in1=dinv2a_sb[:].to_broadcast([128, NT, 16]),
                op=ALU.mult,
            )
            nc.vector.tensor_tensor(out=tmp[:], in0=tmp[:], in1=tmp2[:], op=ALU.add)
            nc.vector.tensor_tensor(out=tmp[:], in0=tmp[:], in1=b1_sb[:], op=ALU.add)
            relu_sb = spool.tile([128, NT, 16], F32)
            nc.scalar.activation(out=relu_sb[:], in_=tmp[:], func=AF.Relu)
            v2_sb = spool.tile([128, NT, 16], BF16)
            nc.vector.tensor_tensor(
                out=v2_sb[:], in0=relu_sb[:],
                in1=dinva_sb[:].to_broadcast([128, NT, 16]), op=ALU.mult,
            )
            nc.sync.dma_start(
                out=relu1.rearrange("(t p) f -> p t f", p=128), in_=relu_sb[:]
            )
            nc.sync.dma_start(out=v2.rearrange("(t p) f -> p t f", p=128), in_=v2_sb[:])
    _legalize_waits(nc)
    return nc


def build_C(NT, nchunks, chunks):
    """s2 -> agg2 -> @W2 + b2 -> log_softmax."""
    nc = bass.Bass()
    g = nc.dram_tensor("g", [128, nchunks, 16], BF16, kind="ExternalInput")
    dstid = nc.dram_tensor("dstid", [128, nchunks], BF16, kind="ExternalInput")
    iota = nc.dram_tensor("iota", [128, GROUP, SC], BF16, kind="ExternalInput")
    relu1 = nc.dram_tensor("relu1", [NT * 128, 16], F32, kind="ExternalInput")
    dinva = nc.dram_tensor("dinva", [128, NT], F32, kind="ExternalInput")
    dinv2a = nc.dram_tensor("dinv2a", [128, NT], F32, kind="ExternalInput")
    b2rep = nc.dram_tensor("b2rep", [128, NT, 16], F32, kind="ExternalInput")
    ident = nc.dram_tensor("ident", [128, 128], F32, kind="ExternalInput")
    W2b = nc.dram_tensor("W2b", [16, 16], BF16, kind="ExternalInput")
    outd = nc.dram_tensor("outd", [NT * 128, 16], F32, kind="ExternalOutput")
    with tile.TileContext(nc) as tc:
        with (
            tc.tile_pool(name="sbuf", bufs=2) as pool,
            tc.tile_pool(name="stat", bufs=1) as spool,
            tc.tile_pool(name="psum", bufs=4, space="PSUM") as pp,
            tc.tile_pool(name="psumt", bufs=2, space="PSUM") as ppt,
        ):
            dstid_sb = spool.tile([128, nchunks], BF16)
            nc.sync.dma_start(out=dstid_sb[:], in_=dstid[:])
            iota_sb = spool.tile([128, GROUP, SC], BF16)
            nc.sync.dma_start(out=iota_sb[:], in_=iota[:])
            r1_sb = spool.tile([128, NT, 16], F32)
            nc.sync.dma_start(
                out=r1_sb[:], in_=relu1.rearrange("(t p) f -> p t f", p=128)
            )
            dinva_sb = spool.tile([128, NT], F32)
            nc.sync.dma_start(out=dinva_sb[:], in_=dinva[:])
            dinv2a_sb = spool.tile([128, NT], F32)
            nc.sync.dma_start(out=dinv2a_sb[:], in_=dinv2a[:])
            b2_sb = spool.tile([128, NT, 16], F32)
            nc.sync.dma_start(out=b2_sb[:], in_=b2rep[:])
            id_sb = spool.tile([128, 128], F32)
            nc.sync.dma_start(out=id_sb[:], in_=ident[:])
            w2_sb = spool.tile([16, 16], BF16)
            nc.sync.dma_start(out=w2_sb[:], in_=W2b[:])
            s_sb = spool.tile([128, NT, 16], F32)

            _emit_segsum(nc, tc, pool, spool, pp, g, dstid_sb, iota_sb, chunks, s_sb, nchunks)

            agg = spool.tile([128, NT, 16], F32)
            tmp2 = spool.tile([128, NT, 16], F32)
            nc.vector.tensor_tensor(
                out=agg[:], in0=s_sb[:], in1=dinva_sb[:].to_broadcast([128, NT, 16]),
                op=ALU.mult,
            )
            nc.vector.tensor_tensor(
                out=tmp2[:], in0=r1_sb[:], in1=dinv2a_sb[:].to_broadcast([128, NT, 16]),
                op=ALU.mult,
            )
            nc.vector.tensor_tensor(out=agg[:], in0=agg[:], in1=tmp2[:], op=ALU.add)

            z_sb = spool.tile([128, NT, 16], F32)
            for tb in range(0, NT, 4):
                w4 = min(4, NT - tb)
                trps = ppt.tile([16, 512], F32, tag="trps")
                for k in range(w4):
                    nc.tensor.transpose(
                        out=trps[:, 128 * k : 128 * (k + 1)],
                        in_=agg[:, tb + k, :],
                        identity=id_sb[:],
                    )
                aggT = pool.tile([16, 512], BF16, tag="aggT")
                nc.scalar.copy(out=aggT[:, : 128 * w4], in_=trps[:, : 128 * w4])
                for k in range(w4):
                    zps = ppt.tile([128, 16], F32, tag="zps")
                    nc.tensor.matmul(
                        out=zps[:],
                        lhsT=aggT[:, 128 * k : 128 * (k + 1)],
                        rhs=w2_sb[:],
                        start=True,
                        stop=True,
                    )
                    nc.scalar.copy(out=z_sb[:, tb + k, :], in_=zps[:])

            nc.vector.tensor_tensor(out=z_sb[:], in0=z_sb[:], in1=b2_sb[:], op=ALU.add)
            m_sb = spool.tile([128, NT], F32)
            nc.vector.tensor_reduce(
                out=m_sb[:], in_=z_sb[:], axis=mybir.AxisListType.X, op=ALU.max
            )
            zc = spool.tile([128, NT, 16], F32)
            nc.vector.tensor_tensor(
                out=zc[:], in0=z_sb[:], in1=m_sb[:].to_broadcast([128, NT, 16]),
                op=ALU.subtract,
            )
            e_sb = spool.tile([128, NT, 16], F32)
            nc.scalar.activation(out=e_sb[:], in_=zc[:], func=AF.Exp)
            ss = spool.tile([128, NT], F32)
            nc.vector.tensor_reduce(
                out=ss[:], in_=e_sb[:], axis=mybir.AxisListType.X, op=ALU.add
            )
            lse = spool.tile([128, NT], F32)
            nc.scalar.activation(out=lse[:], in_=ss[:], func=AF.Ln)
            o_sb = spool.tile([128, NT, 16], F32)
            nc.vector.tensor_tensor(
                out=o_sb[:], in0=zc[:], in1=lse[:].to_broadcast([128, NT, 16]),
                op=ALU.subtract,
            )
            nc.sync.dma_start(out=outd.rearrange("(t p) f -> p t f", p=128), in_=o_sb[:])
    _legalize_waits(nc)
    return nc


# ---------------------------------------------------------------------------
# host side
# ---------------------------------------------------------------------------


def _preprocess(edge_index, n_nodes, per_core):
    """Sort edges by dst, build common-across-cores slot/chunk structure."""
    src = np.asarray(edge_index[0])
    dst = np.asarray(edge_index[1])
    deg = np.bincount(dst, minlength=n_nodes).astype(np.float32) + 1.0
    dinv = 1.0 / np.sqrt(deg)

    order = np.argsort(dst, kind="stable")
    sdst = dst[order]
    ssrc = src[order]

    NT = (per_core + 127) // 128
    padded = NT * 128
    ngroups = padded // GROUP

    bounds = np.searchsorted(sdst, np.arange(N_CORES + 1) * per_core)
    core_grp_cnt = np.zeros((N_CORES, ngroups), np.int64)
    core_edges = []
    for c in range(N_CORES):
        lo, hi = bounds[c], bounds[c + 1]
        ld = sdst[lo:hi] - c * per_core
        grp = ld >> 5
        core_grp_cnt[c] = np.bincount(grp, minlength=ngroups)
        core_edges.append((ld, ssrc[lo:hi]))

    nchunk_g = np.maximum((core_grp_cnt.max(axis=0) + 127) // 128, 1)
    chunk_base = np.concatenate([[0], np.cumsum(nchunk_g)])
    nchunks = int(chunk_base[-1])
    # pad nchunks to a multiple of 4 for tidiness
    chunks = []
    for gi in range(ngroups):
        for k in range(nchunk_g[gi]):
            chunks.append((gi, k == 0, k == nchunk_g[gi] - 1))

    dstid_arrs, sidx_arrs = [], []
    for c in range(N_CORES):
        ld, esrc = core_edges[c]
        grp = ld >> 5
        # rank of each edge within its group (edges sorted by dst => grouped)
        gstart = np.concatenate([[0], np.cumsum(core_grp_cnt[c])])
        rank = np.arange(len(ld)) - np.repeat(gstart[:-1], core_grp_cnt[c])
        slot = chunk_base[grp] * 128 + rank
        nslots = nchunks * 128
        dstid_slots = np.full(nslots, -1.0, np.float32)
        dstid_slots[slot] = (ld & 31).astype(np.float32)
        sidx_slots = np.zeros(nslots, np.int64)
        sidx_slots[slot] = esrc
        dstid_arrs.append(
            dstid_slots.reshape(nchunks, 128).T.astype(NPBF16).copy()
        )
        sidx_arrs.append(sidx_slots.reshape(nchunks, 128).T.copy())
    return dinv, NT, nchunks, chunks, dstid_arrs, sidx_arrs


_CACHE = {}
LAST_TIMES = {}
LAST_HW_NS = None
_TRACE = bool(os.environ.get("KERNEL_TRACE"))


def _sim_ns(nc):
    """Cost-model (CoreSim no-exec) execution time of one dispatch, ns."""
    from concourse.bass_interp import CoreSim

    sim = CoreSim(nc, no_exec=True)
    sim.simulate()
    return int(sim.time)


def _run(nc, in_maps, cores, tag):
    import time as _t

    global LAST_HW_NS
    t0 = _t.time()
    res = run_bass_kernel_spmd(nc, in_maps, core_ids=cores, trace=_TRACE)
    LAST_TIMES[f"disp_{tag}"] = _t.time() - t0
    if res.exec_time_ns is not None:
        LAST_TIMES[f"hw_{tag}_ns"] = res.exec_time_ns
        LAST_HW_NS = (LAST_HW_NS or 0) + res.exec_time_ns
    return res


def _kernel_impl(x, W1, b1, W2, b2, edge_index, n_nodes, per_core):
    x = np.asarray(x, dtype=np.float32)
    W1 = np.asarray(W1, dtype=np.float32)
    b1 = np.asarray(b1, dtype=np.float32)
    W2 = np.asarray(W2, dtype=np.float32)
    b2 = np.asarray(b2, dtype=np.float32)
    edge_index = np.asarray(edge_index)
    fin = x.shape[1]

    import time as _t
    LAST_TIMES.clear()
    _tp = _t.time()
    dinv, NT, nchunks, chunks, dstid_arrs, sidx_arrs = _preprocess(
        edge_index, n_nodes, per_core
    )
    LAST_TIMES["preprocess"] = _t.time() - _tp
    padded = NT * 128
    cores = list(range(N_CORES))

    key = (n_nodes, per_core, nchunks)
    if key not in _CACHE:
        ncA = build_A(NT, fin)
        ncB = build_B(NT, nchunks, chunks)
        ncC = build_C(NT, nchunks, chunks)
        try:
            hw_ns = _sim_ns(ncA) + _sim_ns(ncB) + _sim_ns(ncC)
        except Exception:
            hw_ns = None
        _CACHE[key] = (ncA, ncB, ncC, hw_ns)
    ncA, ncB, ncC, _hw = _CACHE[key]
    global LAST_HW_NS
    LAST_HW_NS = _hw
    LAST_TIMES["build"] = _t.time() - _tp

    # ---- dispatch A: h1 = x @ W1 ----
    FC = fin // 128
    W1bf = W1.astype(NPBF16)
    W1r = W1bf.reshape(FC, 128, 16).transpose(1, 0, 2).copy()  # [128, FC, 16]
    in_A = []
    for c in cores:
        xs = x[c * per_core : (c + 1) * per_core]
        xp = np.zeros((padded, fin), np.float32)
        xp[: xs.shape[0]] = xs
        xTr = (
            xp.T.astype(NPBF16).reshape(FC, 128, padded).transpose(1, 0, 2).copy()
        )  # [128, FC, padded]
        in_A.append({"xT": xTr, "W1b": W1r})
    LAST_TIMES["prep_A"] = _t.time() - _tp
    resA = _run(ncA, in_A, cores, "A")
    h1s = [resA.results[c]["h1"] for c in cores]  # [padded, 16] f32

    # ---- host gather for layer 1 ----
    _tp = _t.time()
    u1 = np.concatenate([h1s[c][:per_core] for c in cores], axis=0)
    u1 *= dinv[:, None]

    # static scale/bias arrays per core
    iota_np = np.ascontiguousarray(
        np.broadcast_to(
            np.arange(GROUP, dtype=np.float32)[:, None], (128, GROUP, SC)
        )
    ).astype(NPBF16)
    ident_np = np.eye(128, dtype=np.float32)
    W2bf = W2.astype(NPBF16)
    dinva_c, dinv2a_c, b1rep, b2rep = [], [], None, None
    for c in cores:
        dv = np.ones(padded, np.float32)
        dv[:per_core] = dinv[c * per_core : (c + 1) * per_core]
        dinva_c.append(dv.reshape(NT, 128).T.copy())
        dinv2a_c.append((dv * dv).reshape(NT, 128).T.copy())
    b1rep = np.broadcast_to(b1, (128, NT, 16)).astype(np.float32).copy()
    b2rep = np.broadcast_to(b2, (128, NT, 16)).astype(np.float32).copy()

    def gath(table, c):
        return table[sidx_arrs[c]].astype(NPBF16)  # [128, nchunks, 16]

    # ---- dispatch B ----
    in_B = []
    for c in cores:
        in_B.append(
            {
                "g": gath(u1, c),
                "dstid": dstid_arrs[c],
                "iota": iota_np,
                "h1": h1s[c],
                "dinva": dinva_c[c],
                "dinv2a": dinv2a_c[c],
                "b1rep": b1rep,
            }
        )
    LAST_TIMES["prep_B"] = _t.time() - _tp
    resB = _run(ncB, in_B, cores, "B")
    relu1s = [resB.results[c]["relu1"] for c in cores]
    v2s = [resB.results[c]["v2"] for c in cores]

    # ---- host gather for layer 2 ----
    _tp = _t.time()
    v2full = np.concatenate(
        [v2s[c][:per_core].astype(np.float32) for c in cores], axis=0
    )

    # ---- dispatch C ----
    in_C = []
    for c in cores:
        in_C.append(
            {
                "g": gath(v2full, c),
                "dstid": dstid_arrs[c],
                "iota": iota_np,
                "relu1": relu1s[c],
                "dinva": dinva_c[c],
                "dinv2a": dinv2a_c[c],
                "b2rep": b2rep,
                "ident": ident_np,
                "W2b": W2bf,
            }
        )
    LAST_TIMES["prep_C"] = _t.time() - _tp
    resC = _run(ncC, in_C, cores, "C")
    out = np.concatenate(
        [resC.results[c]["outd"][:per_core] for c in cores], axis=0
    ).astype(np.float32)
    return out


def kernel(x, W1, b1, W2, b2, edge_index):
    return _kernel_impl(x, W1, b1, W2, b2, edge_index, 100000, 12500)



# revision 10
# speedup vs baseline: 1.2429x; 1.0003x over previous
"""GCN (2-layer, PyG GCNConv semantics) on 8 Trainium2 NeuronCores.

Strategy (dst-shard, graph-parallel):
- Nodes are sharded contiguously across the 8 cores (12500 dsts/core).
- All dense math runs on-device via Bass/Tile in 3 SPMD dispatches:
    A: h1 = x @ W1           (x shipped pre-transposed in bf16, PE matmuls)
    B: s1 = segment-sum of gathered u1 rows over dst groups (PE staircase
       one-hot matmuls built on-device from per-slot dst offsets), fused
       epilogue -> relu1, v2 = dinv*relu1
    C: same segment-sum machinery for layer 2, then @W2 + b2 + log_softmax
- The edge structure (sort order, slot layout, staircase metadata) is
  compile-time constant: it is baked into the instruction stream / tiny
  static inputs at kernel-build time.
- The two per-edge value gathers (u[src] for 3.2M edges) run on the host
  between dispatches: every data-driven gather primitive available in this
  toolchain was measured unusable (indirect DMA ~1.6us/row and 128 rows per
  call; GPSIMD gather ucode unloadable under this walrus build).
"""
import os
import sys
import numpy as np

sys.path.insert(0, "/opt/trn_rl_repo")

import ml_dtypes
import concourse.bass as bass
import concourse.mybir as mybir
import concourse.tile as tile
from concourse.vector_clock import ScopedClock
from concourse.bass_utils import run_bass_kernel_spmd

BF16 = mybir.dt.bfloat16
F32 = mybir.dt.float32
AF = mybir.ActivationFunctionType
ALU = mybir.AluOpType
NPBF16 = ml_dtypes.bfloat16

N_CORES = 8
GROUP = 32          # dsts per staircase group (matmul M)
SC = 64             # chunks per superchunk (is_equal batch)

# ---------------------------------------------------------------------------
# walrus workaround: only ONE sync-wait command per instruction is accepted.
# ---------------------------------------------------------------------------


def _patched_drain_and_barrier(self, tick_clock, wait_clock):
    nc = self.nc
    carrier = nc.sync.nop(nofuse=True, hint="drain_wait_carrier")
    wait_clock.add_sem_waits(carrier.ins, ScopedClock({None: tick_clock.global_clock}))
    si = carrier.ins.sync_info
    waits = list(si.on_wait or []) if si else []
    if len(waits) > 1:
        si.on_wait = waits[:1]
        for i in range(1, len(waits)):
            extra = nc.sync.nop(nofuse=True, hint="drain_wait_carrier")
            extra.ins.sync_info = mybir.SyncInfo(on_wait=waits[i : i + 1], on_update=[])
    nc.sync.drain()
    nc.all_engine_barrier()
    assert self.sems is not None
    popped = nc._tile_sem_poison_stack.pop()
    assert popped is self._sem_poison
    nc.clear_and_free_semaphores(list(self.sems.allocated().values()))
    nc.all_engine_barrier()


tile.TileContext._drain_and_barrier = _patched_drain_and_barrier


def _legalize_waits(nc, max_waits=1):
    n = [0]

    def mk_nop(engine, waits):
        n[0] += 1
        return mybir.InstNoOp(
            name=f"waitnop-{n[0]}",
            engine=engine,
            ins=[],
            outs=[],
            sync_info=mybir.SyncInfo(on_wait=list(waits), on_update=[]),
            text_hint="wait_carrier",
        )

    for f in nc.m.functions:
        for bb in f.blocks:
            out = []
            changed = False
            for inst in bb.instructions:
                si = inst.sync_info
                waits = list(si.on_wait or []) if si else []
                if len(waits) > max_waits:
                    changed = True
                    for i in range(0, len(waits) - max_waits, max_waits):
                        out.append(mk_nop(inst.engine, waits[i : i + max_waits]))
                    si.on_wait = waits[len(waits) - max_waits :]
                out.append(inst)
            if changed:
                bb.instructions = out


# ---------------------------------------------------------------------------
# device kernel builders
# ---------------------------------------------------------------------------


def build_A(NT, FIN=512):
    """h1 = x @ W1 per core. xTr host layout [128, FIN//128, NT*128] bf16."""
    FC = FIN // 128
    nc = bass.Bass()
    xT = nc.dram_tensor("xT", [128, FC, NT * 128], BF16, kind="ExternalInput")
    W1b = nc.dram_tensor("W1b", [128, FC, 16], BF16, kind="ExternalInput")
    h1 = nc.dram_tensor("h1", [NT * 128, 16], F32, kind="ExternalOutput")
    with tile.TileContext(nc) as tc:
        with (
            tc.tile_pool(name="stat", bufs=1) as spool,
            tc.tile_pool(name="psum", bufs=8, space="PSUM") as pp,
        ):
            w1 = spool.tile([128, FC, 16], BF16)
            nc.sync.dma_start(out=w1[:], in_=W1b[:])
            # one whole-tensor load: per-partition contiguous FC*NT*128*2B run
            # (per-tile loads move 256B/partition chunks -> 2x sub-512B DMA
            # penalty; this is one big descriptor per partition instead)
            xsb = spool.tile([128, FC, NT * 128], BF16)
            nc.sync.dma_start(out=xsb[:], in_=xT[:])
            h_sb = spool.tile([128, NT, 16], F32)
            for t in range(NT):
                ps = pp.tile([128, 16], F32, tag="hps")
                for fc in range(FC):
                    nc.tensor.matmul(
                        out=ps[:],
                        lhsT=xsb[:, fc, 128 * t : 128 * (t + 1)],
                        rhs=w1[:, fc, :],
                        start=(fc == 0),
                        stop=(fc == FC - 1),
                    )
                nc.scalar.copy(out=h_sb[:, t, :], in_=ps[:])
            nc.sync.dma_start(
                out=h1.rearrange("(t p) f -> p t f", p=128), in_=h_sb[:]
            )
    _legalize_waits(nc)
    return nc


def _emit_segsum(nc, tc, pool, spool, pp, g_dram, dstid_sb, iota_sb, chunks, s_sb, nchunks):
    """Staircase segment-sum: s_sb[128, NT, 16] f32 <- sum of g rows per dst."""
    nsc = (nchunks + SC - 1) // SC
    ps = None
    for sc in range(nsc):
        cs = sc * SC
        w = min(SC, nchunks - cs)
        g_sc = pool.tile([128, SC, 16], BF16, tag="gsc")
        nc.sync.dma_start(out=g_sc[:, :w, :], in_=g_dram[:, cs : cs + w, :])
        # d-major one-hot [128, GROUP, w]: every operand's last dim is packed
        # 2-byte, so the DVE runs this in its 2x perf mode (the old chunk-major
        # layout put the broadcast on the last dim, forcing full-rate).
        s_all = pool.tile([128, GROUP, SC], BF16, tag="sall")
        nc.vector.tensor_tensor(
            out=s_all[:, :, :w],
            in0=dstid_sb[:, cs : cs + w]
            .rearrange("p (o j) -> p o j", o=1)
            .to_broadcast([128, GROUP, w]),
            in1=iota_sb[:, :, :w],
            op=ALU.is_equal,
        )
        for j in range(w):
            grp, st, sp = chunks[cs + j]
            if st:
                ps = pp.tile([GROUP, 16], F32, tag="ps")
            nc.tensor.matmul(
                out=ps[:],
                lhsT=s_all[:, :, j],
                rhs=g_sc[:, j, :],
                start=st,
                stop=sp,
            )
            if sp:
                po = GROUP * (grp % (128 // GROUP))
                nc.scalar.copy(
                    out=s_sb[po : po + GROUP, grp // (128 // GROUP), :], in_=ps[:]
                )


def build_B(NT, nchunks, chunks):
    """s1 -> agg1 -> relu1, v2."""
    nc = bass.Bass()
    g = nc.dram_tensor("g", [128, nchunks, 16], BF16, kind="ExternalInput")
    dstid = nc.dram_tensor("dstid", [128, nchunks], BF16, kind="ExternalInput")
    iota = nc.dram_tensor("iota", [128, GROUP, SC], BF16, kind="ExternalInput")
    h1 = nc.dram_tensor("h1", [NT * 128, 16], F32, kind="ExternalInput")
    dinva = nc.dram_tensor("dinva", [128, NT], F32, kind="ExternalInput")
    dinv2a = nc.dram_tensor("dinv2a", [128, NT], F32, kind="ExternalInput")
    b1rep = nc.dram_tensor("b1rep", [128, NT, 16], F32, kind="ExternalInput")
    relu1 = nc.dram_tensor("relu1", [NT * 128, 16], F32, kind="ExternalOutput")
    v2 = nc.dram_tensor("v2", [NT * 128, 16], BF16, kind="ExternalOutput")
    with tile.TileContext(nc) as tc:
        with (
            tc.tile_pool(name="sbuf", bufs=3) as pool,
            tc.tile_pool(name="stat", bufs=1) as spool,
            tc.tile_pool(name="psum", bufs=8, space="PSUM") as pp,
        ):
            dstid_sb = spool.tile([128, nchunks], BF16)
            nc.sync.dma_start(out=dstid_sb[:], in_=dstid[:])
            iota_sb = spool.tile([128, GROUP, SC], BF16)
            nc.sync.dma_start(out=iota_sb[:], in_=iota[:])
            h1_sb = spool.tile([128, NT, 16], F32)
            nc.sync.dma_start(out=h1_sb[:], in_=h1.rearrange("(t p) f -> p t f", p=128))
            dinva_sb = spool.tile([128, NT], F32)
            nc.sync.dma_start(out=dinva_sb[:], in_=dinva[:])
            dinv2a_sb = spool.tile([128, NT], F32)
            nc.sync.dma_start(out=dinv2a_sb[:], in_=dinv2a[:])
            b1_sb = spool.tile([128, NT, 16], F32)
            nc.sync.dma_start(out=b1_sb[:], in_=b1rep[:])
            s_sb = spool.tile([128, NT, 16], F32)

            _emit_segsum(nc, tc, pool, spool, pp, g, dstid_sb, iota_sb, chunks, s_sb, nchunks)

            tmp = spool.tile([128, NT, 16], F32)
            tmp2 = spool.tile([128, NT, 16], F32)
            nc.vector.tensor_tensor(
                out=tmp[:], in0=s_sb[:], in1=dinva_sb[:].to_broadcast([128, NT, 16]),
                op=ALU.mult,
            )
            nc.vector.tensor_tensor(
                out=tmp2[:], in0=h1_sb[:], in1=dinv2a_sb[:].to_broadcast([128, NT, 16]),
                op=ALU.mult,
            )
            nc.vector.tensor_tensor(out=tmp[:], in0=tmp[:], in1=tmp2[:], op=ALU.add)
            nc.vector.tensor_tensor(out=tmp[:], in0=tmp[:], in1=b1_sb[:], op=ALU.add)
            relu_sb = spool.tile([128, NT, 16], F32)
            nc.scalar.activation(out=relu_sb[:], in_=tmp[:], func=AF.Relu)
            v2_sb = spool.tile([128, NT, 16], BF16)
            nc.vector.tensor_tensor(
                out=v2_sb[:], in0=relu_sb[:],
                in1=dinva_sb[:].to_broadcast([128, NT, 16]), op=ALU.mult,
            )
            nc.sync.dma_start(
                out=relu1.rearrange("(t p) f -> p t f", p=128), in_=relu_sb[:]
            )
            nc.sync.dma_start(out=v2.rearrange("(t p) f -> p t f", p=128), in_=v2_sb[:])
    _legalize_waits(nc)
    return nc


def build_C(NT, nchunks, chunks):
    """s2 -> agg2 -> @W2 + b2 -> log_softmax."""
    nc = bass.Bass()
    g = nc.dram_tensor("g", [128, nchunks, 16], BF16, kind="ExternalInput")
    dstid = nc.dram_tensor("dstid", [128, nchunks], BF16, kind="ExternalInput")
    iota = nc.dram_tensor("iota", [128, GROUP, SC], BF16, kind="ExternalInput")
    relu1 = nc.dram_tensor("relu1", [NT * 128, 16], F32, kind="ExternalInput")
    dinva = nc.dram_tensor("dinva", [128, NT], F32, kind="ExternalInput")
    dinv2a = nc.dram_tensor("dinv2a", [128, NT], F32, kind="ExternalInput")
    b2rep = nc.dram_tensor("b2rep", [128, NT, 16], F32, kind="ExternalInput")
    ident = nc.dram_tensor("ident", [128, 128], F32, kind="ExternalInput")
    W2b = nc.dram_tensor("W2b", [16, 16], BF16, kind="ExternalInput")
    outd = nc.dram_tensor("outd", [NT * 128, 16], F32, kind="ExternalOutput")
    with tile.TileContext(nc) as tc:
        with (
            tc.tile_pool(name="sbuf", bufs=3) as pool,
            tc.tile_pool(name="stat", bufs=1) as spool,
            tc.tile_pool(name="psum", bufs=4, space="PSUM") as pp,
            tc.tile_pool(name="psumt", bufs=2, space="PSUM") as ppt,
        ):
            dstid_sb = spool.tile([128, nchunks], BF16)
            nc.sync.dma_start(out=dstid_sb[:], in_=dstid[:])
            iota_sb = spool.tile([128, GROUP, SC], BF16)
            nc.sync.dma_start(out=iota_sb[:], in_=iota[:])
            r1_sb = spool.tile([128, NT, 16], F32)
            nc.sync.dma_start(
                out=r1_sb[:], in_=relu1.rearrange("(t p) f -> p t f", p=128)
            )
            dinva_sb = spool.tile([128, NT], F32)
            nc.sync.dma_start(out=dinva_sb[:], in_=dinva[:])
            dinv2a_sb = spool.tile([128, NT], F32)
            nc.sync.dma_start(out=dinv2a_sb[:], in_=dinv2a[:])
            b2_sb = spool.tile([128, NT, 16], F32)
            nc.sync.dma_start(out=b2_sb[:], in_=b2rep[:])
            id_sb = spool.tile([128, 128], F32)
            nc.sync.dma_start(out=id_sb[:], in_=ident[:])
            w2_sb = spool.tile([16, 16], BF16)
            nc.sync.dma_start(out=w2_sb[:], in_=W2b[:])
            s_sb = spool.tile([128, NT, 16], F32)

            _emit_segsum(nc, tc, pool, spool, pp, g, dstid_sb, iota_sb, chunks, s_sb, nchunks)

            agg = spool.tile([128, NT, 16], F32)
            tmp2 = spool.tile([128, NT, 16], F32)
            nc.vector.tensor_tensor(
                out=agg[:], in0=s_sb[:], in1=dinva_sb[:].to_broadcast([128, NT, 16]),
                op=ALU.mult,
            )
            nc.vector.tensor_tensor(
                out=tmp2[:], in0=r1_sb[:], in1=dinv2a_sb[:].to_broadcast([128, NT, 16]),
                op=ALU.mult,
            )
            nc.vector.tensor_tensor(out=agg[:], in0=agg[:], in1=tmp2[:], op=ALU.add)

            z_sb = spool.tile([128, NT, 16], F32)
            for tb in range(0, NT, 4):
                w4 = min(4, NT - tb)
                trps = ppt.tile([16, 512], F32, tag="trps")
                for k in range(w4):
                    nc.tensor.transpose(
                        out=trps[:, 128 * k : 128 * (k + 1)],
                        in_=agg[:, tb + k, :],
                        identity=id_sb[:],
                    )
                aggT = pool.tile([16, 512], BF16, tag="aggT")
                nc.scalar.copy(out=aggT[:, : 128 * w4], in_=trps[:, : 128 * w4])
                for k in range(w4):
                    zps = ppt.tile([128, 16], F32, tag="zps")
                    nc.tensor.matmul(
                        out=zps[:],
                        lhsT=aggT[:, 128 * k : 128 * (k + 1)],
                        rhs=w2_sb[:],
                        start=True,
                        stop=True,
                    )
                    nc.scalar.copy(out=z_sb[:, tb + k, :], in_=zps[:])

            nc.vector.tensor_tensor(out=z_sb[:], in0=z_sb[:], in1=b2_sb[:], op=ALU.add)
            m_sb = spool.tile([128, NT], F32)
            nc.vector.tensor_reduce(
                out=m_sb[:], in_=z_sb[:], axis=mybir.AxisListType.X, op=ALU.max
            )
            zc = spool.tile([128, NT, 16], F32)
            nc.vector.tensor_tensor(
                out=zc[:], in0=z_sb[:], in1=m_sb[:].to_broadcast([128, NT, 16]),
                op=ALU.subtract,
            )
            e_sb = spool.tile([128, NT, 16], F32)
            nc.scalar.activation(out=e_sb[:], in_=zc[:], func=AF.Exp)
            ss = spool.tile([128, NT], F32)
            nc.vector.tensor_reduce(
                out=ss[:], in_=e_sb[:], axis=mybir.AxisListType.X, op=ALU.add
            )
            lse = spool.tile([128, NT], F32)
            nc.scalar.activation(out=lse[:], in_=ss[:], func=AF.Ln)
            o_sb = spool.tile([128, NT, 16], F32)
            nc.vector.tensor_tensor(
                out=o_sb[:], in0=zc[:], in1=lse[:].to_broadcast([128, NT, 16]),
                op=ALU.subtract,
            )
            nc.sync.dma_start(out=outd.rearrange("(t p) f -> p t f", p=128), in_=o_sb[:])
    _legalize_waits(nc)
    return nc


# ---------------------------------------------------------------------------
# host side
# ---------------------------------------------------------------------------


def _preprocess(edge_index, n_nodes, per_core):
    """Sort edges by dst, build common-across-cores slot/chunk structure."""
    src = np.asarray(edge_index[0])
    dst = np.asarray(edge_index[1])
    deg = np.bincount(dst, minlength=n_nodes).astype(np.float32) + 1.0
    dinv = 1.0 / np.sqrt(deg)

    order = np.argsort(dst, kind="stable")
    sdst = dst[order]
    ssrc = src[order]

    NT = (per_core + 127) // 128
    padded = NT * 128
    ngroups = padded // GROUP

    bounds = np.searchsorted(sdst, np.arange(N_CORES + 1) * per_core)
    core_grp_cnt = np.zeros((N_CORES, ngroups), np.int64)
    core_edges = []
    for c in range(N_CORES):
        lo, hi = bounds[c], bounds[c + 1]
        ld = sdst[lo:hi] - c * per_core
        grp = ld >> 5
        core_grp_cnt[c] = np.bincount(grp, minlength=ngroups)
        core_edges.append((ld, ssrc[lo:hi]))

    nchunk_g = np.maximum((core_grp_cnt.max(axis=0) + 127) // 128, 1)
    chunk_base = np.concatenate([[0], np.cumsum(nchunk_g)])
    nchunks = int(chunk_base[-1])
    # pad nchunks to a multiple of 4 for tidiness
    chunks = []
    for gi in range(ngroups):
        for k in range(nchunk_g[gi]):
            chunks.append((gi, k == 0, k == nchunk_g[gi] - 1))

    dstid_arrs, sidx_arrs = [], []
    for c in range(N_CORES):
        ld, esrc = core_edges[c]
        grp = ld >> 5
        # rank of each edge within its group (edges sorted by dst => grouped)
        gstart = np.concatenate([[0], np.cumsum(core_grp_cnt[c])])
        rank = np.arange(len(ld)) - np.repeat(gstart[:-1], core_grp_cnt[c])
        slot = chunk_base[grp] * 128 + rank
        nslots = nchunks * 128
        dstid_slots = np.full(nslots, -1.0, np.float32)
        dstid_slots[slot] = (ld & 31).astype(np.float32)
        sidx_slots = np.zeros(nslots, np.int64)
        sidx_slots[slot] = esrc
        dstid_arrs.append(
            dstid_slots.reshape(nchunks, 128).T.astype(NPBF16).copy()
        )
        sidx_arrs.append(sidx_slots.reshape(nchunks, 128).T.copy())
    return dinv, NT, nchunks, chunks, dstid_arrs, sidx_arrs


_CACHE = {}
LAST_TIMES = {}
LAST_HW_NS = None
_TRACE = bool(os.environ.get("KERNEL_TRACE"))


def _sim_ns(nc):
    """Cost-model (CoreSim no-exec) execution time of one dispatch, ns."""
    from concourse.bass_interp import CoreSim

    sim = CoreSim(nc, no_exec=True)
    sim.simulate()
    return int(sim.time)


def _run(nc, in_maps, cores, tag):
    import time as _t

    global LAST_HW_NS
    t0 = _t.time()
    res = run_bass_kernel_spmd(nc, in_maps, core_ids=cores, trace=_TRACE)
    LAST_TIMES[f"disp_{tag}"] = _t.time() - t0
    if res.exec_time_ns is not None:
        LAST_TIMES[f"hw_{tag}_ns"] = res.exec_time_ns
        LAST_HW_NS = (LAST_HW_NS or 0) + res.exec_time_ns
    return res


def _kernel_impl(x, W1, b1, W2, b2, edge_index, n_nodes, per_core):
    x = np.asarray(x, dtype=np.float32)
    W1 = np.asarray(W1, dtype=np.float32)
    b1 = np.asarray(b1, dtype=np.float32)
    W2 = np.asarray(W2, dtype=np.float32)
    b2 = np.asarray(b2, dtype=np.float32)
    edge_index = np.asarray(edge_index)
    fin = x.shape[1]

    import time as _t
    LAST_TIMES.clear()
    _tp = _t.time()
    dinv, NT, nchunks, chunks, dstid_arrs, sidx_arrs = _preprocess(
        edge_index, n_nodes, per_core
    )
    LAST_TIMES["preprocess"] = _t.time() - _tp
    padded = NT * 128
    cores = list(range(N_CORES))

    key = (n_nodes, per_core, nchunks)
    if key not in _CACHE:
        ncA = build_A(NT, fin)
        ncB = build_B(NT, nchunks, chunks)
        ncC = build_C(NT, nchunks, chunks)
        try:
            hw_ns = _sim_ns(ncA) + _sim_ns(ncB) + _sim_ns(ncC)
        except Exception:
            hw_ns = None
        _CACHE[key] = (ncA, ncB, ncC, hw_ns)
    ncA, ncB, ncC, _hw = _CACHE[key]
    global LAST_HW_NS
    LAST_HW_NS = _hw
    LAST_TIMES["build"] = _t.time() - _tp

    # ---- dispatch A: h1 = x @ W1 ----
    FC = fin // 128
    W1bf = W1.astype(NPBF16)
    W1r = W1bf.reshape(FC, 128, 16).transpose(1, 0, 2).copy()  # [128, FC, 16]
    in_A = []
    for c in cores:
        xs = x[c * per_core : (c + 1) * per_core]
        xp = np.zeros((padded, fin), np.float32)
        xp[: xs.shape[0]] = xs
        xTr = (
            xp.T.astype(NPBF16).reshape(FC, 128, padded).transpose(1, 0, 2).copy()
        )  # [128, FC, padded]
        in_A.append({"xT": xTr, "W1b": W1r})
    LAST_TIMES["prep_A"] = _t.time() - _tp
    resA = _run(ncA, in_A, cores, "A")
    h1s = [resA.results[c]["h1"] for c in cores]  # [padded, 16] f32

    # ---- host gather for layer 1 ----
    _tp = _t.time()
    u1 = np.concatenate([h1s[c][:per_core] for c in cores], axis=0)
    u1 *= dinv[:, None]

    # static scale/bias arrays per core
    iota_np = np.ascontiguousarray(
        np.broadcast_to(
            np.arange(GROUP, dtype=np.float32)[:, None], (128, GROUP, SC)
        )
    ).astype(NPBF16)
    ident_np = np.eye(128, dtype=np.float32)
    W2bf = W2.astype(NPBF16)
    dinva_c, dinv2a_c, b1rep, b2rep = [], [], None, None
    for c in cores:
        dv = np.ones(padded, np.float32)
        dv[:per_core] = dinv[c * per_core : (c + 1) * per_core]
        dinva_c.append(dv.reshape(NT, 128).T.copy())
        dinv2a_c.append((dv * dv).reshape(NT, 128).T.copy())
    b1rep = np.broadcast_to(b1, (128, NT, 16)).astype(np.float32).copy()
    b2rep = np.broadcast_to(b2, (128, NT, 16)).astype(np.float32).copy()

    def gath(table, c):
        return table[sidx_arrs[c]].astype(NPBF16)  # [128, nchunks, 16]

    # ---- dispatch B ----
    in_B = []
    for c in cores:
        in_B.append(
            {
                "g": gath(u1, c),
                "dstid": dstid_arrs[c],
                "iota": iota_np,
                "h1": h1s[c],
                "dinva": dinva_c[c],
                "dinv2a": dinv2a_c[c],
                "b1rep": b1rep,
            }
        )
    LAST_TIMES["prep_B"] = _t.time() - _tp
    resB = _run(ncB, in_B, cores, "B")
    relu1s = [resB.results[c]["relu1"] for c in cores]
    v2s = [resB.results[c]["v2"] for c in cores]

    # ---- host gather for layer 2 ----
    _tp = _t.time()
    v2full = np.concatenate(
        [v2s[c][:per_core].astype(np.float32) for c in cores], axis=0
    )

    # ---- dispatch C ----
    in_C = []
    for c in cores:
        in_C.append(
            {
                "g": gath(v2full, c),
                "dstid": dstid_arrs[c],
                "iota": iota_np,
                "relu1": relu1s[c],
                "dinva": dinva_c[c],
                "dinv2a": dinv2a_c[c],
                "b2rep": b2rep,
                "ident": ident_np,
                "W2b": W2bf,
            }
        )
    LAST_TIMES["prep_C"] = _t.time() - _tp
    resC = _run(ncC, in_C, cores, "C")
    out = np.concatenate(
        [resC.results[c]["outd"][:per_core] for c in cores], axis=0
    ).astype(np.float32)
    return out


def kernel(x, W1, b1, W2, b2, edge_index):
    return _kernel_impl(x, W1, b1, W2, b2, edge_index, 100000, 12500)

